# revision 11
# baseline (speedup 1.0000x reference)
"""Weighted-BCE loss kernel for Trainium2 (8 NeuronCores, SPMD data-parallel).

Reference math (torch-style BCELoss with class-balancing weights):
    n   = len(x), s = sum(gt)
    w0  = n / (2*(n-s)),  w1 = n / (2*s)
    L1  = max(log(x),     -100)
    L0  = max(log1p(-x),  -100)
    loss = mean( where(gt==0, w0, w1) * -(gt*L1 + (1-gt)*L0) )

The weights depend only on the GLOBAL positive count s, so the loss
decomposes into 4 global sums computed shard-locally:
    A = sum(gt * L1),  B = sum(gt * L0),  C = sum(L0),  s = sum(gt)
    loss = -( A/(2s) + (C-B)/(2(n-s)) )

Each core processes a 1/8 shard laid out [128 partitions, 16384 free].
Engine budget (measured): ACT 0.95 ns/col/pass, DVE 1.12 ns/col/pass,
DMA-in ~5.7us per 2048-col x+gt tile pair.  The 5 compute passes
(2 Ln on ACT, A/B STTs on DVE, S) must be balanced UNIFORMLY per tile
(front/back-loading starves one engine early and bottlenecks the other
late), so each tile's S pass is split by columns: ~5/8 on ACT (Copy+
accum), ~3/8 on DVE (STT+accum), landing both engines at ~5.3us/tile,
just under the DMA pace:
  - All input DMAs stream through the single SP HWDGE ring, interleaved
    x_i, gt_i in consumption order; deep pools keep the ring ahead.
  - ScalarE (ACT): Ln(x); Ln(1-x) (affine scale=-1,bias=1) whose
    accum_out produces C free; Copy+accum over the tile's first
    S_ACT_FRAC columns of gt.
  - VectorE (DVE): S-STT over the remaining gt columns placed FIRST
    (needs only gt - fills DVE's wait on lnx), then the A and B
    clamp+mult+accum STTs.  All f32 (bf16/int32 mixing measured slower).
  - Small first tile starts ACT earlier; small last tile shortens the
    tail.
Host gathers the partial-sum columns (A|B|C|Sa|Sd groups) from all 8
cores and finishes the tiny all-reduce + final scalar math in float64.
"""

import numpy as np
from contextlib import ExitStack

import concourse.bass as bass
import concourse.bacc as bacc
import concourse.mybir as mybir
import concourse.tile as tile
from concourse.alu_op_type import AluOpType
from concourse.bass_utils import run_bass_kernel_spmd

N_TOTAL = 16777216
N_CORES = 8
PER_CORE = N_TOTAL // N_CORES   # 2097152
P = 128
FD = PER_CORE // P              # 16384 free elements per partition
TILE_SIZES = [1024] + [2048] * 7 + [1024]
assert sum(TILE_SIZES) == FD
NT = len(TILE_SIZES)
LOG_CLAMP = -100.0
# fraction (in 1/8ths) of each tile's S columns summed on ACT; rest on DVE
S_ACT_EIGHTHS = 5

# Optional instrumentation knobs for a driver script (harness never sets them).
TRACE = False
LAST_RESULTS = None

_NC_CACHE = None


def _build():
    f32 = mybir.dt.float32
    i32 = mybir.dt.int32
    Ln = mybir.ActivationFunctionType.Ln
    Copy = mybir.ActivationFunctionType.Copy

    nc = bacc.Bacc("TRN2")
    x_in = nc.declare_dram_parameter("x", [P, FD], f32, isOutput=False)
    g_in = nc.declare_dram_parameter("gt", [P, FD], i32, isOutput=False)
    # packed output: column groups [A | B | C | S_act | S_dve], NT each
    out_all = nc.declare_dram_parameter("out_all", [P, 5 * NT], f32, isOutput=True)

    with tile.TileContext(nc) as tc, ExitStack() as ctx:
        xp = ctx.enter_context(tc.tile_pool(name="xp", bufs=5))
        gp = ctx.enter_context(tc.tile_pool(name="gp", bufs=5))
        lp = ctx.enter_context(tc.tile_pool(name="lp", bufs=3))
        jp = ctx.enter_context(tc.tile_pool(name="jp", bufs=1))
        accp = ctx.enter_context(tc.tile_pool(name="accp", bufs=1))

        acc = accp.tile([P, 5 * NT], f32)

        def col(group, i):
            j = group * NT + i
            return acc[:, j : j + 1]

        off = 0
        for i, tfd in enumerate(TILE_SIZES):
            sl = slice(off, off + tfd)
            off += tfd
            xt = xp.tile([P, tfd], f32, tag="xt")
            gt_t = gp.tile([P, tfd], i32, tag="gt")
            # single SP ring, consumption order: tile i's data lands before
            # tile i+1's, each transfer at full aggregate queue bandwidth
            nc.sync.dma_start(xt[:], x_in[:, sl])
            nc.sync.dma_start(gt_t[:], g_in[:, sl])

            lnx = lp.tile([P, tfd], f32, tag="lnx")
            ln1 = lp.tile([P, tfd], f32, tag="ln1")
            nc.scalar.activation(lnx[:], xt[:], Ln)
            nc.scalar.activation(
                ln1[:], xt[:], Ln, bias=1.0, scale=-1.0,
                accum_out=col(2, i),
            )

            ha = tfd * S_ACT_EIGHTHS // 8  # S columns summed on ACT
            junk3 = jp.tile([P, tfd], f32, tag="junk3")
            # S (DVE part): needs only gt - before A/B to fill DVE's bubble
            nc.vector.scalar_tensor_tensor(
                junk3[:, ha:], gt_t[:, ha:], 0.0, gt_t[:, ha:],
                AluOpType.mult, AluOpType.add,
                accum_out=col(4, i),
            )

            junk = jp.tile([P, tfd], f32, tag="junk")
            nc.vector.scalar_tensor_tensor(
                junk[:], lnx[:], LOG_CLAMP, gt_t[:],
                AluOpType.max, AluOpType.mult,
                accum_out=col(0, i),
            )
            junk2 = jp.tile([P, tfd], f32, tag="junk")
            nc.vector.scalar_tensor_tensor(
                junk2[:], ln1[:], LOG_CLAMP, gt_t[:],
                AluOpType.max, AluOpType.mult,
                accum_out=col(1, i),
            )

            # S (ACT part)
            nc.scalar.activation(
                junk3[:, :ha], gt_t[:, :ha], Copy, accum_out=col(3, i)
            )

        nc.sync.dma_start(out_all[:], acc[:])

    nc.compile()
    return nc


def get_nc():
    global _NC_CACHE
    if _NC_CACHE is None:
        _NC_CACHE = _build()
    return _NC_CACHE


def make_in_maps(x, gt):
    x = np.ascontiguousarray(np.asarray(x, dtype=np.float32).reshape(-1))
    gt = np.ascontiguousarray(np.asarray(gt, dtype=np.int32).reshape(-1))
    assert x.shape == (N_TOTAL,) and gt.shape == (N_TOTAL,)
    in_maps = []
    for c in range(N_CORES):
        sl = slice(c * PER_CORE, (c + 1) * PER_CORE)
        in_maps.append({
            "x": x[sl].reshape(P, FD),
            "gt": gt[sl].reshape(P, FD),
        })
    return in_maps


def combine(results):
    """All-reduce the per-core partial sums and finish the loss formula."""
    A = B = C = S = 0.0
    for r in results:
        o = r["out_all"].astype(np.float64)
        A += o[:, 0 * NT : 1 * NT].sum()
        B += o[:, 1 * NT : 2 * NT].sum()
        C += o[:, 2 * NT : 3 * NT].sum()
        S += o[:, 3 * NT : 5 * NT].sum()   # S_act + S_dve
    n = float(N_TOTAL)
    result = -(A / (2.0 * S) + (C - B) / (2.0 * (n - S)))
    return np.array(result, dtype=np.float32)


def kernel(x, gt):
    global LAST_RESULTS
    nc = get_nc()
    in_maps = make_in_maps(x, gt)
    br = run_bass_kernel_spmd(nc, in_maps, list(range(N_CORES)))
    LAST_RESULTS = br
    return combine(br.results)
